# revision 32
# baseline (speedup 1.0000x reference)
"""Bass/Trainium2 kernel for nn_BatchLoreAttentionLayer.

Reference math (per batch item b, X = embeddings[b] in [L=128, D=256]):
    Q = X q_w^T + q_b ; K = X k_w^T + k_b
    S = Q K^T / sqrt(D) ; S[:, padded] = -inf
    attn = softmax_m(S) ; attended = attn X
    out = tanh( (valid^T attended) / cnt )

Algebraic restructure used here:
    S = X A X^T / sqrt(D) + row_const(l) + s(m),  A = q_w^T k_w
    row_const drops inside softmax; s = X (k_w^T q_b) / sqrt(D).
    out_b = tanh( w X ),  w[m] = sum_l g[l] E[l,m],
    E = exp(S + rowbias),  g[l] = valid[b,l] / (rowsum[l] * cnt_b)
so the [L,D] attended matrix is never materialized: per item the device does
    Yt = A^T Xt (batched 4 items),  S = Yt^T Xt,  S += ones^T rowbias (mask),
    E,rowsum = ACT exp,  w = E^T g,  outT[:,b] = X^T w, tanh at chunk end.

Sharding: pure data-parallel over B across 8 cores (256 items/core).
Host prep: cast X to bf16 (plus a pre-transposed copy), fold sqrt(D), mask,
bias and counts into A / rowbias / valid tensors.
"""

import sys
from contextlib import ExitStack

import numpy as np
import ml_dtypes

sys.path.insert(0, "/opt/trn_rl_repo")

import concourse.bass as bass  # noqa: E402
import concourse.mybir as mybir  # noqa: E402
import concourse.tile as tile  # noqa: E402
from concourse import bacc  # noqa: E402
from concourse.bass import ts  # noqa: E402
from concourse.bass_utils import run_bass_kernel_spmd  # noqa: E402

B, L, D = 2048, 128, 256
NCORES = 8
BPC = B // NCORES  # items per core
CHUNK = 128        # items per output accumulation chunk
GRP = 4            # items per Yt matmul group (N = GRP*L = 512)

F32 = mybir.dt.float32
BF16 = mybir.dt.bfloat16
AF = mybir.ActivationFunctionType

_CACHE = {}


def build_bass():
    nc = bacc.Bacc(None, target_bir_lowering=False)
    # pre-grouped on host: one dense DMA per 4-item group per view
    xl = nc.declare_dram_parameter("xl", [BPC // GRP, 128, GRP * D], BF16, isOutput=False)
    xt = nc.declare_dram_parameter(
        "xt", [BPC // GRP, 128, 2 * GRP * L], BF16, isOutput=False
    )
    rb = nc.declare_dram_parameter("rb", [BPC, L], BF16, isOutput=False)
    vt = nc.declare_dram_parameter("vt", [L, BPC], F32, isOutput=False)
    aw = nc.declare_dram_parameter("aw", [D, D], BF16, isOutput=False)
    outT = nc.declare_dram_parameter("outT", [2, 128, BPC], F32, isOutput=True)

    build_body(nc, xl, xt, rb, vt, aw, outT)
    nc.finalize()
    return nc


def build_body(nc, xl, xt, rb, vt, aw, outT):
    with tile.TileContext(nc) as tc, ExitStack() as ctx:
        singles = ctx.enter_context(tc.tile_pool(name="singles", bufs=1))
        io = ctx.enter_context(tc.tile_pool(name="io", bufs=6))
        work = ctx.enter_context(tc.tile_pool(name="work", bufs=5))
        small = ctx.enter_context(tc.tile_pool(name="small", bufs=10))
        ps_yt = ctx.enter_context(tc.tile_pool(name="ps_yt", bufs=3, space="PSUM"))
        ps_s = ctx.enter_context(tc.tile_pool(name="ps_s", bufs=3, space="PSUM"))
        ps_w = ctx.enter_context(tc.tile_pool(name="ps_w", bufs=1, space="PSUM"))
        ps_o = ctx.enter_context(tc.tile_pool(name="ps_o", bufs=1, space="PSUM"))

        # ---- one-time loads ----
        a_sb = singles.tile([128, 2, D], BF16)  # [d_sub, d_tile, e]
        nc.sync.dma_start(out=a_sb[:, 0, :], in_=aw[0:128, :])
        nc.sync.dma_start(out=a_sb[:, 1, :], in_=aw[128:256, :])
        vt_sb = singles.tile([128, BPC], F32)   # valid/cnt, [l, b]
        nc.sync.dma_start(out=vt_sb, in_=vt[:, :])
        ones_sb = singles.tile([1, 128], BF16)
        nc.vector.memset(ones_sb, 1.0)
        # rowbias rows, chunked loads onto partition 0
        rb_sb = singles.tile([1, BPC, L], BF16)
        nc.sync.dma_start(
            out=rb_sb, in_=rb.rearrange("(o b) l -> o b l", o=1)
        )

        n_chunks = BPC // CHUNK
        for c in range(n_chunks):
            oT_ps = ps_o.tile([128, 2, CHUNK], F32, tag="oT")
            wcol_ps = ps_w.tile([128, CHUNK], F32, tag="w")
            for g in range(CHUNK // GRP):
                i0 = c * CHUNK + g * GRP
                col0 = i0 % CHUNK
                # ---- load group of 4 items ----
                xt4f = io.tile([128, 2 * GRP * L], BF16, tag="xt4")
                xl4f = io.tile([128, GRP * D], BF16, tag="xl4")
                gi = i0 // GRP
                nc.sync.dma_start(out=xt4f, in_=xt[gi])
                nc.gpsimd.dma_start(out=xl4f, in_=xl[gi])
                xt4 = xt4f.rearrange("p (t jl) -> p t jl", t=2)  # [d_sub, d_tile, 4l]
                xl4 = xl4f.rearrange("p (j d) -> p j d", j=GRP)  # [l, item, d]

                # ---- Yt = A^T Xt for 4 items at once (N=512) ----
                yt0_ps = ps_yt.tile([128, GRP * L], F32, tag="yt")
                yt1_ps = ps_yt.tile([128, GRP * L], F32, tag="yt")
                for e2, ytp in ((0, yt0_ps), (1, yt1_ps)):
                    for d2 in range(2):
                        nc.tensor.matmul(
                            out=ytp,
                            lhsT=a_sb[:, d2, ts(e2, 128)],
                            rhs=xt4[:, d2, :],
                            start=(d2 == 0),
                            stop=(d2 == 1),
                        )
                yt_sb = work.tile([128, 2, GRP * L], BF16, tag="yt_sb")
                nc.scalar.activation(
                    out=yt_sb[:, 0, :], in_=yt0_ps, func=AF.Copy
                )
                nc.vector.tensor_copy(out=yt_sb[:, 1, :], in_=yt1_ps)

                # ---- S for all 4 items into one PSUM tile ----
                s4_ps = ps_s.tile([128, GRP, L], F32, tag="s4")
                for j in range(GRP):
                    for e2 in range(2):
                        nc.tensor.matmul(
                            out=s4_ps[:, j, :],
                            lhsT=yt_sb[:, e2, ts(j, L)],
                            rhs=xt4[:, e2, ts(j, L)],
                            start=(e2 == 0),
                            stop=False,
                        )
                    nc.tensor.matmul(
                        out=s4_ps[:, j, :],
                        lhsT=ones_sb,
                        rhs=rb_sb[0:1, i0 + j, :],
                        start=False,
                        stop=True,
                    )
                # ---- E = exp(S), one ACT op per quad ----
                e4_sb = work.tile([128, GRP, L], BF16, tag="e4")
                nc.scalar.activation(out=e4_sb, in_=s4_ps, func=AF.Exp)
                rs4 = small.tile([128, GRP], F32, tag="rs4")
                nc.vector.reduce_sum(out=rs4, in_=e4_sb, axis=mybir.AxisListType.X)
                rinv4 = small.tile([128, GRP], F32, tag="rinv4")
                nc.vector.reciprocal(out=rinv4, in_=rs4)
                g4 = small.tile([128, GRP], BF16, tag="g4")
                nc.vector.tensor_mul(g4, rinv4, vt_sb[:, i0 : i0 + GRP])
                # ---- w = E^T g per item, into chunk-wide PSUM columns ----
                for j in range(GRP):
                    nc.tensor.matmul(
                        out=wcol_ps[:, col0 + j : col0 + j + 1],
                        lhsT=e4_sb[:, j, :],
                        rhs=g4[:, j : j + 1],
                        start=True,
                        stop=True,
                    )
                w4_sb = small.tile([128, GRP], BF16, tag="w4")
                nc.vector.tensor_copy(
                    out=w4_sb, in_=wcol_ps[:, col0 : col0 + GRP]
                )
                # ---- outT[:, i] = X^T w ----
                for j in range(GRP):
                    for dh in range(2):
                        nc.tensor.matmul(
                            out=oT_ps[:, dh, col0 + j : col0 + j + 1],
                            lhsT=xl4[:, j, ts(dh, 128)],
                            rhs=w4_sb[:, j : j + 1],
                            start=True,
                            stop=True,
                        )
            # ---- tanh + store chunk ----
            oT_sb = work.tile([128, 2, CHUNK], F32, tag="oT_sb")
            nc.scalar.activation(out=oT_sb, in_=oT_ps, func=AF.Tanh)
            for dh in range(2):
                nc.sync.dma_start(
                    out=outT[dh, :, c * CHUNK : (c + 1) * CHUNK],
                    in_=oT_sb[:, dh, :],
                )


def _get_nc():
    if "nc" not in _CACHE:
        _CACHE["nc"] = build_bass()
    return _CACHE["nc"]


def prep_inputs(embeddings, padding_mask, q_w, q_b, k_w, k_b):
    """Host-side shard prep: dtype casts, weight folding, mask/count folding."""
    emb = np.asarray(embeddings, np.float32)
    mask = np.asarray(padding_mask)
    q_w = np.asarray(q_w, np.float32)
    k_w = np.asarray(k_w, np.float32)
    q_b = np.asarray(q_b, np.float32)
    scale = 1.0 / np.sqrt(np.float32(D))

    A = (q_w.T @ k_w) * scale                      # [D, D] (d, e)
    v = (k_w.T @ q_b) * scale                      # [D]
    rowbias = np.where(mask, np.float32(-1e9), np.float32(0.0))
    if np.any(v):
        rowbias = rowbias + emb @ v                # s(m) term (q_b != 0 case)
    valid = (~mask).astype(np.float32)             # [B, L]
    cnt = np.maximum(valid.sum(1, keepdims=True), 1.0)
    vt_full = (valid / cnt).T.astype(np.float32)   # [L, B]

    bf = ml_dtypes.bfloat16
    emb16 = emb.astype(bf)
    # xl groups: [B/4, L(=p), 4, D] flattened to [B/4, 128, 1024]
    xlg = (
        emb16.reshape(B // GRP, GRP, L, D)
        .transpose(0, 2, 1, 3)
        .reshape(B // GRP, 128, GRP * D)
    )
    # xt groups: [B/4, d_sub(128), d_tile(2), 4*L] flat [B/4, 128, 1024]
    xtg = (
        emb16.transpose(0, 2, 1)
        .reshape(B // GRP, GRP, 2, 128, L)
        .transpose(0, 3, 2, 1, 4)
        .reshape(B // GRP, 128, 2 * GRP * L)
    )
    xlg = np.ascontiguousarray(xlg)
    xtg = np.ascontiguousarray(xtg)
    rb16 = rowbias.astype(bf)
    A16 = A.astype(bf)

    gpc = BPC // GRP
    in_maps = []
    for c in range(NCORES):
        sl = slice(c * BPC, (c + 1) * BPC)
        in_maps.append(
            {
                "xl": xlg[c * gpc : (c + 1) * gpc],
                "xt": xtg[c * gpc : (c + 1) * gpc],
                "rb": rb16[sl],
                "vt": np.ascontiguousarray(vt_full[:, sl]),
                "aw": A16,
            }
        )
    return in_maps


def _make_exec():
    """Build the shard_map'd PJRT executable once (mirrors
    bass2jax.run_bass_via_pjrt, but returns a reusable callable)."""
    import jax
    from jax.sharding import Mesh, PartitionSpec
    from jax.experimental.shard_map import shard_map
    from concourse import bass2jax, mybir as _mybir

    nc = _get_nc()
    bass2jax.install_neuronx_cc_hook()
    partition_name = nc.partition_id_tensor.name if nc.partition_id_tensor else None
    in_names, out_names, out_avals, zero_outs = [], [], [], []
    for alloc in nc.m.functions[0].allocations:
        if not isinstance(alloc, _mybir.MemoryLocationSet):
            continue
        name = alloc.memorylocations[0].name
        if alloc.kind == "ExternalInput":
            if name != partition_name:
                in_names.append(name)
        elif alloc.kind == "ExternalOutput":
            shape = tuple(alloc.tensor_shape)
            dtype = _mybir.dt.np(alloc.dtype)
            out_names.append(name)
            out_avals.append(jax.core.ShapedArray(shape, dtype))
            zero_outs.append(np.zeros(shape, dtype))
    n_params = len(in_names)
    in_names_full = in_names + out_names
    if partition_name is not None:
        in_names_full.append(partition_name)

    def _body(*args):
        operands = list(args)
        if partition_name is not None:
            operands.append(bass2jax.partition_id_tensor())
        outs = bass2jax._bass_exec_p.bind(
            *operands,
            out_avals=tuple(out_avals),
            in_names=tuple(in_names_full),
            out_names=tuple(out_names),
            lowering_input_output_aliases=(),
            sim_require_finite=True,
            sim_require_nnan=True,
            nc=nc,
        )
        return tuple(outs)

    devices = jax.devices()[:NCORES]
    mesh = Mesh(np.asarray(devices), ("core",))
    n_outs = len(out_names)
    sharded = jax.jit(
        shard_map(
            _body,
            mesh=mesh,
            in_specs=(PartitionSpec("core"),) * (n_params + n_outs),
            out_specs=(PartitionSpec("core"),) * n_outs,
            check_rep=False,
        ),
        donate_argnums=tuple(range(n_params, n_params + n_outs)),
        keep_unused=True,
    )

    def run(in_maps, n_iters=1, timings=None):
        import time as _t

        concat_in = [
            np.concatenate([np.asarray(in_maps[c][nm]) for c in range(NCORES)], axis=0)
            for nm in in_names
        ]
        placed = [jax.device_put(a) for a in concat_in]
        zo = [np.concatenate([z] * NCORES, axis=0) for z in zero_outs]
        outs = None
        for _ in range(n_iters):
            zplaced = [jax.device_put(z) for z in zo]
            for p in placed + zplaced:
                p.block_until_ready()
            t0 = _t.perf_counter()
            outs = sharded(*placed, *zplaced)
            for o in outs:
                o.block_until_ready()
            if timings is not None:
                timings.append(_t.perf_counter() - t0)
        res = []
        for c in range(NCORES):
            d = {}
            for i, nm in enumerate(out_names):
                full = np.asarray(outs[i])
                per = full.shape[0] // NCORES
                d[nm] = full[c * per : (c + 1) * per]
            res.append(d)
        return res

    return run


def _get_runner():
    if "run" not in _CACHE:
        _CACHE["run"] = _make_exec()
    return _CACHE["run"]


def kernel(embeddings, padding_mask, q_w, q_b, k_w, k_b, _n_iters=1, _timings=None):
    in_maps = prep_inputs(embeddings, padding_mask, q_w, q_b, k_w, k_b)
    results = _get_runner()(in_maps, n_iters=_n_iters, timings=_timings)
    out = np.empty((B, D), np.float32)
    for c in range(NCORES):
        oT = np.asarray(results[c]["outT"], np.float32)  # [2, 128, BPC]
        out[c * BPC : (c + 1) * BPC] = oT.reshape(D, BPC).T
    return out


if __name__ == "__main__":
    ref_inputs = {
        "embeddings": np.random.randn(B, L, D).astype(np.float32),
        "padding_mask": np.random.rand(B, L) < 0.3,
        "q_w": np.random.randn(D, D).astype(np.float32) * 0.06,
        "q_b": np.zeros(D, np.float32),
        "k_w": np.random.randn(D, D).astype(np.float32) * 0.06,
        "k_b": np.zeros(D, np.float32),
    }
    out = kernel(**ref_inputs)
    print(out.shape, out.dtype)
